# revision 7
# baseline (speedup 1.0000x reference)
"""Trainium2 Bass kernel for nn_Dense_BinaryLayer (binary-weight dense layer).

out = x @ Wb + b, where Wb = binarize(W) in {-1, +1}.

Data-parallel over 8 NeuronCores (2048 rows of x each, W/b replicated, no
collectives).  Host-side staging is layout/dtype only (transpose + bf16
round-to-nearest); every multiply-accumulate runs on device.

Per core, all-bf16 single pass (measured: fp8 DoubleRow runs at 157 TF/s =
2x bf16 per MAC, so an fp8 hi/lo split that doubles the MACs is a wash and
only adds DVE latency; bf16 x feeds the PE straight from DMA):
  - x arrives k-major as bf16 [1024, 2048] split in four 512-row i-blocks
    across the three DMA rings; no on-chip preprocessing of x at all.
  - W arrives as bf16 in four j-column slices; DVE binarizes each slice in
    two 2x-rate tensor_scalar ops: m = (W > 2^-24) in {0,1} (maps the one
    W==0 element to -1 like the reference round-half-even), wb = 2m-1.
    j-sliced chunks mean the first output-column tile has its full-depth
    weights after only 512 KB of W traffic.
  - PE: W-stationary bf16 matmuls ([k=128, j=128] x [k=128, i=512])
    accumulate the 8 k-tiles into [j=128, i=512] PSUM banks; output is
    computed transposed [j, i].
  - Eviction fuses the per-partition bias while casting to bf16, split
    between DVE (tensor_scalar add) and ScalarE (activation Identity) so
    neither engine gates the PE; stores stream on the sync ring.
  - Host detransposes/upcasts the [1024, 2048] bf16 outputs (layout only).
"""
import sys

sys.path.insert(0, "/opt/trn_rl_repo")

import numpy as np

N_TOTAL = 16384
D_IN = 1024
D_OUT = 1024
N_CORES = 8
ROWS = N_TOTAL // N_CORES      # 2048 rows (i) per core
P = 128
K_TILES = D_IN // P            # 8 k-tiles of 128
J_TILES = D_OUT // P           # 8 output-column tiles
I_BLK = 512                    # moving free dim per matmul
I_BLKS = ROWS // I_BLK         # 4 i-blocks
J_CHUNK = 256                  # W arrives in 4 j-column slices
BIN_THRESH = 2.0 ** -24

_cached = {}


def _build():
    import concourse.tile as tile
    from concourse import bacc, mybir

    f32 = mybir.dt.float32
    bf16 = mybir.dt.bfloat16
    TS = mybir.AluOpType
    ACT = mybir.ActivationFunctionType

    nc = bacc.Bacc()
    xt_d = nc.declare_dram_parameter("xT", [D_IN, ROWS], bf16, isOutput=False)
    w_d = nc.declare_dram_parameter("W", [D_IN, D_OUT], bf16, isOutput=False)
    b_d = nc.declare_dram_parameter("b", [D_OUT], f32, isOutput=False)
    o_d = nc.declare_dram_parameter("outT", [D_OUT, ROWS], bf16, isOutput=True)

    with tile.TileContext(nc) as tc:
        with (
            tc.tile_pool(name="const", bufs=1) as const,
            tc.tile_pool(name="wmp", bufs=2) as wmp,
            tc.tile_pool(name="outp", bufs=4) as outp,
            tc.tile_pool(name="pso", bufs=8, space="PSUM") as pso,
        ):
            xt_ap = xt_d[:].rearrange("(kt p) i -> p kt i", p=P)
            w_ap = w_d[:].rearrange("(kt p) j -> p kt j", p=P)

            xbf = const.tile([P, K_TILES, ROWS], bf16, tag="xbf")
            wraw = const.tile([P, K_TILES, D_OUT], bf16, tag="wraw")
            wbb = const.tile([P, K_TILES, D_OUT], bf16, tag="wbb")
            b_sb = const.tile([P, J_TILES], f32, tag="bsb")

            # --- DMA in, spread over the three rings -----------------------
            # sync (HWDGE, first bytes ~8.8us): W k-halves (2KB lines), then
            # the output stores
            nc.sync.dma_start(wraw[:, 0:4, :], w_ap[:, 0:4, :])
            nc.sync.dma_start(wraw[:, 4:8, :], w_ap[:, 4:8, :])
            # scalar (HWDGE, ~10.5us): bias + x i-block 0, nothing behind it
            nc.scalar.dma_start(b_sb[:], b_d[:].rearrange("(jt p) -> p jt", p=P))
            nc.scalar.dma_start(xbf[:, :, 0:512], xt_ap[:, :, 0:512])
            # gpsimd (SWDGE, ~12.9us): x i-blocks 1..3
            for ib in range(1, I_BLKS):
                sl = slice(ib * I_BLK, (ib + 1) * I_BLK)
                nc.gpsimd.dma_start(xbf[:, :, sl], xt_ap[:, :, sl])

            # --- binarize W on DVE per k-half (both ops 2x-rate bf16) ------
            for kc in range(2):
                sl = slice(kc * 4, (kc + 1) * 4)
                wm = wmp.tile([P, 4, D_OUT], bf16, tag="wm", name=f"wm_{kc}")
                nc.vector.tensor_scalar(
                    wm[:], wraw[:, sl, :], BIN_THRESH, None, TS.is_gt,
                )
                nc.vector.tensor_scalar(
                    wbb[:, sl, :], wm[:], 2.0, 1.0, TS.mult, TS.subtract,
                )

            # --- PE: bf16 W-stationary, [j,i] output; evict with bias ------
            def evict(ib, jt, ps):
                osb = outp.tile([P, I_BLK], bf16, tag="osb",
                                name=f"o_{ib}_{jt}")
                if (ib * J_TILES + jt) % 2 == 0:
                    nc.scalar.activation(
                        osb[:], ps[:], ACT.Identity,
                        bias=b_sb[:, jt:jt + 1], scale=1.0,
                    )
                else:
                    nc.vector.tensor_scalar(
                        osb[:], ps[:], b_sb[:, jt:jt + 1], None, TS.add,
                    )
                nc.sync.dma_start(
                    o_d[jt * P:(jt + 1) * P, ib * I_BLK:(ib + 1) * I_BLK],
                    osb[:],
                )

            def burst(ps, jt, kts, i0, start, stop):
                for kt in kts:
                    nc.tensor.matmul(
                        ps[:],
                        wbb[:, kt, jt * P:(jt + 1) * P],
                        xbf[:, kt, i0:i0 + I_BLK],
                        start=start and kt == kts[0],
                        stop=stop and kt == kts[-1],
                    )

            # i-block 0 in two k-passes across all 8 PSUM banks, so matmuls
            # start as soon as the first W k-half is binarized
            ps0 = [pso.tile([P, I_BLK], f32, tag="ps", name=f"ps_0_{jt}")
                   for jt in range(J_TILES)]
            for jt in range(J_TILES):
                burst(ps0[jt], jt, [0, 1, 2, 3], 0, start=True, stop=False)
            for jt in range(J_TILES):
                burst(ps0[jt], jt, [4, 5, 6, 7], 0, start=False, stop=True)
                evict(0, jt, ps0[jt])

            for ib in range(1, I_BLKS):
                i0 = ib * I_BLK
                for jt in range(J_TILES):
                    ps = pso.tile([P, I_BLK], f32, tag="ps",
                                  name=f"ps_{ib}_{jt}")
                    burst(ps, jt, list(range(K_TILES)), i0,
                          start=True, stop=True)
                    evict(ib, jt, ps)

    nc.compile()
    nc.finalize()
    return nc


def kernel(x, W, b):
    import ml_dtypes
    from concourse.bass_utils import run_bass_kernel_spmd

    if "nc" not in _cached:
        _cached["nc"] = _build()
    nc = _cached["nc"]

    x = np.asarray(x, dtype=np.float32)
    W_bf = np.ascontiguousarray(
        np.asarray(W, dtype=np.float32).astype(ml_dtypes.bfloat16))
    b = np.ascontiguousarray(np.asarray(b, dtype=np.float32))

    in_maps = [
        {
            # per-core shard of x, k-major + bf16 (layout/dtype staging only)
            "xT": np.ascontiguousarray(
                x[c * ROWS:(c + 1) * ROWS].T.astype(ml_dtypes.bfloat16)),
            "W": W_bf,
            "b": b,
        }
        for c in range(N_CORES)
    ]
    res = run_bass_kernel_spmd(nc, in_maps, list(range(N_CORES)))
    out = np.concatenate(
        [res.results[c]["outT"].T for c in range(N_CORES)], axis=0)
    return out.astype(np.float32)


# revision 9
# speedup vs baseline: 1.0843x; 1.0843x over previous
"""Trainium2 Bass kernel for nn_Dense_BinaryLayer (binary-weight dense layer).

out = x @ Wb + b, where Wb = binarize(W) in {-1, +1}.

Data-parallel over 8 NeuronCores (2048 rows of x each, W/b replicated, no
collectives).  Host-side staging is layout/dtype only (transpose + bf16
round-to-nearest); every multiply-accumulate runs on device.

Per core, all-bf16 single pass (measured: fp8 DoubleRow runs at 157 TF/s =
2x bf16 per MAC, so an fp8 hi/lo split that doubles the MACs is a wash and
only adds DVE latency; bf16 x feeds the PE straight from DMA):
  - x arrives k-major as bf16 [1024, 2048] split in four 512-row i-blocks
    across the three DMA rings; no on-chip preprocessing of x at all.
  - W arrives as bf16 in four j-column slices; DVE binarizes each slice in
    two 2x-rate tensor_scalar ops: m = (W > 2^-24) in {0,1} (maps the one
    W==0 element to -1 like the reference round-half-even), wb = 2m-1.
    j-sliced chunks mean the first output-column tile has its full-depth
    weights after only 512 KB of W traffic.
  - PE: W-stationary bf16 matmuls ([k=128, j=128] x [k=128, i=512])
    accumulate the 8 k-tiles into [j=128, i=512] PSUM banks; output is
    computed transposed [j, i].
  - Eviction fuses the per-partition bias while casting to bf16, split
    between DVE (tensor_scalar add) and ScalarE (activation Identity) so
    neither engine gates the PE; stores stream on the sync ring.
  - Host detransposes/upcasts the [1024, 2048] bf16 outputs (layout only).
"""
import sys

sys.path.insert(0, "/opt/trn_rl_repo")

import numpy as np

N_TOTAL = 16384
D_IN = 1024
D_OUT = 1024
N_CORES = 8
ROWS = N_TOTAL // N_CORES      # 2048 rows (i) per core
P = 128
K_TILES = D_IN // P            # 8 k-tiles of 128
J_TILES = D_OUT // P           # 8 output-column tiles
I_BLK = 512                    # moving free dim per matmul
I_BLKS = ROWS // I_BLK         # 4 i-blocks
J_CHUNK = 256                  # W arrives in 4 j-column slices
BIN_THRESH = 2.0 ** -24

_cached = {}


def _build():
    import concourse.tile as tile
    from concourse import bacc, mybir

    f32 = mybir.dt.float32
    bf16 = mybir.dt.bfloat16
    TS = mybir.AluOpType
    ACT = mybir.ActivationFunctionType

    nc = bacc.Bacc()
    xt_d = nc.declare_dram_parameter("xT", [D_IN, ROWS], bf16, isOutput=False)
    w_d = nc.declare_dram_parameter("W", [D_IN, D_OUT], bf16, isOutput=False)
    b_d = nc.declare_dram_parameter("b", [D_OUT], f32, isOutput=False)
    o_d = nc.declare_dram_parameter("outT", [D_OUT, ROWS], bf16, isOutput=True)

    with tile.TileContext(nc) as tc:
        with (
            tc.tile_pool(name="const", bufs=1) as const,
            tc.tile_pool(name="wmp", bufs=2) as wmp,
            tc.tile_pool(name="outp", bufs=4) as outp,
            tc.tile_pool(name="pso", bufs=8, space="PSUM") as pso,
        ):
            xt_ap = xt_d[:].rearrange("(kt p) i -> p kt i", p=P)
            w_ap = w_d[:].rearrange("(kt p) j -> p kt j", p=P)

            xbf = const.tile([P, K_TILES, ROWS], bf16, tag="xbf")
            wraw = const.tile([P, K_TILES, D_OUT], bf16, tag="wraw")
            wbb = const.tile([P, K_TILES, D_OUT], bf16, tag="wbb")
            b_sb = const.tile([P, J_TILES], f32, tag="bsb")

            # --- DMA in ----------------------------------------------------
            # All startup-critical input rides the sync HWDGE ring (the only
            # fast one: ~350GB/s from ~8.5us; the scalar HWDGE ring crawls at
            # ~100GB/s and SWDGE starts ~12.5us): W k-quarter 0, x i-block 0,
            # W k-quarters 1..3, then the output stores.
            nc.sync.dma_start(wraw[:, 0:2, :], w_ap[:, 0:2, :])
            nc.sync.dma_start(xbf[:, :, 0:512], xt_ap[:, :, 0:512])
            for kc in range(1, 4):
                sl = slice(2 * kc, 2 * kc + 2)
                nc.sync.dma_start(wraw[:, sl, :], w_ap[:, sl, :])
            # scalar (slow): bias + x i-block 3 (not needed until ~55us)
            nc.scalar.dma_start(b_sb[:], b_d[:].rearrange("(jt p) -> p jt", p=P))
            nc.scalar.dma_start(xbf[:, :, 1536:2048], xt_ap[:, :, 1536:2048])
            # gpsimd (SWDGE): x i-blocks 1, 2
            nc.gpsimd.dma_start(xbf[:, :, 512:1024], xt_ap[:, :, 512:1024])
            nc.gpsimd.dma_start(xbf[:, :, 1024:1536], xt_ap[:, :, 1024:1536])

            # --- binarize W on DVE per k-quarter (both ops 2x-rate bf16) ---
            for kc in range(4):
                sl = slice(2 * kc, 2 * kc + 2)
                wm = wmp.tile([P, 2, D_OUT], bf16, tag="wm", name=f"wm_{kc}")
                nc.vector.tensor_scalar(
                    wm[:], wraw[:, sl, :], BIN_THRESH, None, TS.is_gt,
                )
                nc.vector.tensor_scalar(
                    wbb[:, sl, :], wm[:], 2.0, 1.0, TS.mult, TS.subtract,
                )

            # --- PE: bf16 W-stationary, [j,i] output; evict with bias ------
            def evict(ib, jt, ps):
                osb = outp.tile([P, I_BLK], bf16, tag="osb",
                                name=f"o_{ib}_{jt}")
                if (ib * J_TILES + jt) % 2 == 0:
                    nc.scalar.activation(
                        osb[:], ps[:], ACT.Identity,
                        bias=b_sb[:, jt:jt + 1], scale=1.0,
                    )
                else:
                    nc.vector.tensor_scalar(
                        osb[:], ps[:], b_sb[:, jt:jt + 1], None, TS.add,
                    )
                nc.sync.dma_start(
                    o_d[jt * P:(jt + 1) * P, ib * I_BLK:(ib + 1) * I_BLK],
                    osb[:],
                )

            def burst(ps, jt, kts, i0, start, stop):
                for kt in kts:
                    nc.tensor.matmul(
                        ps[:],
                        wbb[:, kt, jt * P:(jt + 1) * P],
                        xbf[:, kt, i0:i0 + I_BLK],
                        start=start and kt == kts[0],
                        stop=stop and kt == kts[-1],
                    )

            # i-block 0 in four k-passes across all 8 PSUM banks, so matmuls
            # chase the W k-quarters as they land and binarize
            ps0 = [pso.tile([P, I_BLK], f32, tag="ps", name=f"ps_0_{jt}")
                   for jt in range(J_TILES)]
            for kc in range(4):
                for jt in range(J_TILES):
                    burst(ps0[jt], jt, [2 * kc, 2 * kc + 1], 0,
                          start=kc == 0, stop=kc == 3)
                    if kc == 3:
                        evict(0, jt, ps0[jt])

            for ib in range(1, I_BLKS):
                i0 = ib * I_BLK
                for jt in range(J_TILES):
                    ps = pso.tile([P, I_BLK], f32, tag="ps",
                                  name=f"ps_{ib}_{jt}")
                    burst(ps, jt, list(range(K_TILES)), i0,
                          start=True, stop=True)
                    evict(ib, jt, ps)

    nc.compile()
    nc.finalize()
    return nc


def kernel(x, W, b):
    import ml_dtypes
    from concourse.bass_utils import run_bass_kernel_spmd

    if "nc" not in _cached:
        _cached["nc"] = _build()
    nc = _cached["nc"]

    x = np.asarray(x, dtype=np.float32)
    W_bf = np.ascontiguousarray(
        np.asarray(W, dtype=np.float32).astype(ml_dtypes.bfloat16))
    b = np.ascontiguousarray(np.asarray(b, dtype=np.float32))

    in_maps = [
        {
            # per-core shard of x, k-major + bf16 (layout/dtype staging only)
            "xT": np.ascontiguousarray(
                x[c * ROWS:(c + 1) * ROWS].T.astype(ml_dtypes.bfloat16)),
            "W": W_bf,
            "b": b,
        }
        for c in range(N_CORES)
    ]
    res = run_bass_kernel_spmd(nc, in_maps, list(range(N_CORES)))
    out = np.concatenate(
        [res.results[c]["outT"].T for c in range(N_CORES)], axis=0)
    return out.astype(np.float32)


# revision 12
# speedup vs baseline: 1.1584x; 1.0684x over previous
"""Trainium2 Bass kernel for nn_Dense_BinaryLayer (binary-weight dense layer).

out = x @ Wb + b, where Wb = binarize(W) in {-1, +1}.

Data-parallel over 8 NeuronCores (2048 rows of x each, W/b replicated, no
collectives).  Host-side staging is layout/dtype only (transpose + bf16
round-to-nearest); every multiply-accumulate runs on device.

Per core, all-bf16 single pass (measured: fp8 DoubleRow runs at 157 TF/s =
2x bf16 per MAC, so an fp8 hi/lo split that doubles the MACs is a wash and
only adds DVE latency; bf16 x feeds the PE straight from DMA):
  - x arrives k-major as bf16 [1024, 2048] split in four 512-row i-blocks
    across the three DMA rings; no on-chip preprocessing of x at all.
  - W arrives as bf16 in four j-column slices; DVE binarizes each slice in
    two 2x-rate tensor_scalar ops: m = (W > 2^-24) in {0,1} (maps the one
    W==0 element to -1 like the reference round-half-even), wb = 2m-1.
    j-sliced chunks mean the first output-column tile has its full-depth
    weights after only 512 KB of W traffic.
  - PE: W-stationary bf16 matmuls ([k=128, j=128] x [k=128, i=512])
    accumulate the 8 k-tiles into [j=128, i=512] PSUM banks; output is
    computed transposed [j, i].
  - Eviction fuses the per-partition bias while casting to bf16, split
    between DVE (tensor_scalar add) and ScalarE (activation Identity) so
    neither engine gates the PE; stores stream on the sync ring.
  - Host detransposes/upcasts the [1024, 2048] bf16 outputs (layout only).
"""
import sys

sys.path.insert(0, "/opt/trn_rl_repo")

import numpy as np

N_TOTAL = 16384
D_IN = 1024
D_OUT = 1024
N_CORES = 8
ROWS = N_TOTAL // N_CORES      # 2048 rows (i) per core
P = 128
K_TILES = D_IN // P            # 8 k-tiles of 128
J_TILES = D_OUT // P           # 8 output-column tiles
I_BLK = 512                    # moving free dim per matmul
I_BLKS = ROWS // I_BLK         # 4 i-blocks
J_CHUNK = 256                  # W arrives in 4 j-column slices
BIN_THRESH = 2.0 ** -24

_cached = {}


def _build():
    import concourse.tile as tile
    from concourse import bacc, mybir

    f32 = mybir.dt.float32
    bf16 = mybir.dt.bfloat16
    TS = mybir.AluOpType
    ACT = mybir.ActivationFunctionType

    nc = bacc.Bacc()
    xt_d = nc.declare_dram_parameter("xT", [D_IN, ROWS], bf16, isOutput=False)
    w_d = nc.declare_dram_parameter("W", [D_IN, D_OUT], bf16, isOutput=False)
    b_d = nc.declare_dram_parameter("b", [D_OUT], f32, isOutput=False)
    o_d = nc.declare_dram_parameter("outT", [D_OUT, ROWS], bf16, isOutput=True)

    with tile.TileContext(nc) as tc:
        with (
            tc.tile_pool(name="const", bufs=1) as const,
            tc.tile_pool(name="wmp", bufs=2) as wmp,
            tc.tile_pool(name="outp", bufs=6) as outp,
            tc.tile_pool(name="pso", bufs=8, space="PSUM") as pso,
        ):
            xt_ap = xt_d[:].rearrange("(kt p) i -> p kt i", p=P)
            w_ap = w_d[:].rearrange("(kt p) j -> p kt j", p=P)

            xbf = const.tile([P, K_TILES, ROWS], bf16, tag="xbf")
            wraw = const.tile([P, K_TILES, D_OUT], bf16, tag="wraw")
            wbb = const.tile([P, K_TILES, D_OUT], bf16, tag="wbb")
            b_sb = const.tile([P, J_TILES], f32, tag="bsb")

            # --- DMA in ----------------------------------------------------
            # All startup-critical input rides the sync HWDGE ring (the only
            # fast one: ~350GB/s from ~8.5us; the scalar HWDGE ring crawls at
            # ~100GB/s and SWDGE starts ~12.5us): W k-quarter 0, x i-block 0,
            # W k-quarters 1..3, then the output stores.
            # Per-queue DMA is only ~100-220GB/s, so spread: W k-quarters
            # 0,1 on sync (earliest ring) and 2,3 on scalar; all x i-blocks
            # on the fastest ring (gpsimd SWDGE, ~220GB/s); stores alternate
            # sync/scalar behind the input loads.
            nc.sync.dma_start(wraw[:, 0:2, :], w_ap[:, 0:2, :])
            nc.sync.dma_start(wraw[:, 2:4, :], w_ap[:, 2:4, :])
            nc.scalar.dma_start(b_sb[:], b_d[:].rearrange("(jt p) -> p jt", p=P))
            nc.scalar.dma_start(wraw[:, 4:6, :], w_ap[:, 4:6, :])
            nc.scalar.dma_start(wraw[:, 6:8, :], w_ap[:, 6:8, :])
            for ib in range(I_BLKS):
                sl = slice(ib * I_BLK, (ib + 1) * I_BLK)
                nc.gpsimd.dma_start(xbf[:, :, sl], xt_ap[:, :, sl])

            # --- binarize W on DVE per k-quarter (both ops 2x-rate bf16) ---
            for kc in range(4):
                sl = slice(2 * kc, 2 * kc + 2)
                wm = wmp.tile([P, 2, D_OUT], bf16, tag="wm", name=f"wm_{kc}")
                nc.vector.tensor_scalar(
                    wm[:], wraw[:, sl, :], BIN_THRESH, None, TS.is_gt,
                )
                nc.vector.tensor_scalar(
                    wbb[:, sl, :], wm[:], 2.0, 1.0, TS.mult, TS.subtract,
                )

            # --- PE: bf16 W-stationary, [j,i] output; evict with bias ------
            def evict(ib, jt, ps):
                osb = outp.tile([P, I_BLK], bf16, tag="osb",
                                name=f"o_{ib}_{jt}")
                if (ib * J_TILES + jt) % 2 == 0:
                    nc.scalar.activation(
                        osb[:], ps[:], ACT.Identity,
                        bias=b_sb[:, jt:jt + 1], scale=1.0,
                    )
                else:
                    nc.vector.tensor_scalar(
                        osb[:], ps[:], b_sb[:, jt:jt + 1], None, TS.add,
                    )
                ring = nc.sync if (ib * J_TILES + jt) % 2 == 0 else nc.scalar
                ring.dma_start(
                    o_d[jt * P:(jt + 1) * P, ib * I_BLK:(ib + 1) * I_BLK],
                    osb[:],
                )

            def burst(ps, jt, kts, i0, start, stop):
                for kt in kts:
                    nc.tensor.matmul(
                        ps[:],
                        wbb[:, kt, jt * P:(jt + 1) * P],
                        xbf[:, kt, i0:i0 + I_BLK],
                        start=start and kt == kts[0],
                        stop=stop and kt == kts[-1],
                    )

            # i-block 0 in four k-passes across all 8 PSUM banks, so matmuls
            # chase the W k-quarters as they land and binarize
            ps0 = [pso.tile([P, I_BLK], f32, tag="ps", name=f"ps_0_{jt}")
                   for jt in range(J_TILES)]
            for kc in range(4):
                for jt in range(J_TILES):
                    burst(ps0[jt], jt, [2 * kc, 2 * kc + 1], 0,
                          start=kc == 0, stop=kc == 3)
                    if kc == 3:
                        evict(0, jt, ps0[jt])

            for ib in range(1, I_BLKS):
                i0 = ib * I_BLK
                for jt in range(J_TILES):
                    ps = pso.tile([P, I_BLK], f32, tag="ps",
                                  name=f"ps_{ib}_{jt}")
                    burst(ps, jt, list(range(K_TILES)), i0,
                          start=True, stop=True)
                    evict(ib, jt, ps)

    nc.compile()
    nc.finalize()
    return nc


def kernel(x, W, b):
    import ml_dtypes
    from concourse.bass_utils import run_bass_kernel_spmd

    if "nc" not in _cached:
        _cached["nc"] = _build()
    nc = _cached["nc"]

    x = np.asarray(x, dtype=np.float32)
    W_bf = np.ascontiguousarray(
        np.asarray(W, dtype=np.float32).astype(ml_dtypes.bfloat16))
    b = np.ascontiguousarray(np.asarray(b, dtype=np.float32))

    in_maps = [
        {
            # per-core shard of x, k-major + bf16 (layout/dtype staging only)
            "xT": np.ascontiguousarray(
                x[c * ROWS:(c + 1) * ROWS].T.astype(ml_dtypes.bfloat16)),
            "W": W_bf,
            "b": b,
        }
        for c in range(N_CORES)
    ]
    res = run_bass_kernel_spmd(nc, in_maps, list(range(N_CORES)))
    out = np.concatenate(
        [res.results[c]["outT"].T for c in range(N_CORES)], axis=0)
    return out.astype(np.float32)
